# revision 9
# baseline (speedup 1.0000x reference)
"""Trainium2 Bass kernel for nn_DiracGraphConv (GNN edge-softmax message passing).

V4 design (8 NeuronCores, SPMD, no collectives):
  - Shard edges by destination-node slab: core k owns rows
    [k*12500, (k+1)*12500). Per-core output slabs are disjoint; the full
    output is a host-side concatenation (row un-permute).
  - ONE SWDGE descriptor per edge (the previous kernel used three):
    a transposed dma_gather of 256B rows from a bf16 [zh | x] node table
    (zh = z/||z|| precomputed host-side), giving zh_col^T / x_col^T
    directly in SBUF. gpsimd descriptor generation (~7ns/desc, the hard
    bottleneck of this problem) is therefore minimized.
  - The segment-sum (edge softmax numerator/denominator) is done on the
    PE with a mask trick instead of dma_scatter_add:
      * rows are packed into NWIN windows of 64 rows (host bin-packing,
        balanced so every (window, col-group) cell holds <= 256 edges =
        2 subchunks of 128 -- a compile-time uniform schedule).
      * M2[e, r] = zh_col[e] . zh_win[r] via matmul (lhsT = gathered
        zh^T slice, rhs = per-window zh_win^T kept resident in SBUF).
      * masked[e, r] = (rowloc[e] == r) * exp(alpha*M2 - |alpha|); the
        exp shift is valid by softmax shift-invariance.
      * PSUM [64, 65] += masked^T @ [x_col | 1] accumulates messages and
        denominator per window; drained into an SBUF accumulator.
  - x_col in natural layout comes from dma_start_transpose (XBAR maps
    token t -> (partition t%128, slot t//128), matching the gather).
  - Final phase: out = (msgs/(denom+eps)) @ W^T + b via PE transpose +
    matmul with [W^T; b], from SBUF, then one DMA out per core.
"""

import sys

sys.path.insert(0, "/opt/trn_rl_repo")

from dataclasses import dataclass

import numpy as np
import ml_dtypes

from concourse import bacc, bass, mybir, tile
from concourse.library_config import mlp as MLP_LIB
from concourse.masks import make_identity

P = 128
F32 = mybir.dt.float32
BF16 = mybir.dt.bfloat16
I16 = mybir.dt.int16
EPS_DENOM = 1e-9

N_NODES = 100000
N_EDGES = 1600000
D = 64
DD = 128           # [zh | x] row elems (bf16) = 256B
N_CORES = 8
NPC = 12500        # nodes per core
NGRP = 4           # col groups (int16 gather index limit)
GRP = 25000        # nodes per col group
W = 64             # rows per window
CELL = 256         # max edges per (window, group) cell = 2 subchunks
STILE = 14         # slots per super-tile (2 gather instrs of 896)
GB = 896           # tokens per gather instruction


@dataclass(frozen=True)
class Cfg:
    nwin: int = 210  # windows per core; nwin % 7 == 0

    @property
    def nslotg(self) -> int:
        return self.nwin * 2          # subchunk slots per group stream

    @property
    def ntokg(self) -> int:
        return self.nslotg * P        # tokens per group stream

    @property
    def ntok(self) -> int:
        return self.ntokg * NGRP

    @property
    def acc_rows(self) -> int:
        return self.nwin * W


def build_program(cfg: Cfg, alpha: float):
    nwin = cfg.nwin
    nslotg = cfg.nslotg
    n_stile = nslotg // STILE

    nc = bacc.Bacc(
        "TRN2", target_bir_lowering=False, debug=False, num_swdge_queues=4
    )

    tabs = [
        nc.dram_tensor(f"tab{g}", [GRP, DD], BF16, kind="ExternalInput").ap()
        for g in range(NGRP)
    ]
    zhw = nc.dram_tensor("zhw", [D, nwin, W], BF16, kind="ExternalInput").ap()
    cidx = nc.dram_tensor(
        "cidx", [P, cfg.ntok // 16], I16, kind="ExternalInput"
    ).ap()
    rowloc = nc.dram_tensor(
        "rowloc", [P, cfg.ntok // P], BF16, kind="ExternalInput"
    ).ap()
    iota = nc.dram_tensor("iota", [P, W], BF16, kind="ExternalInput").ap()
    wb = nc.dram_tensor("wb", [D + 1, D], F32, kind="ExternalInput").ap()
    out = nc.dram_tensor("out", [cfg.acc_rows, D], F32, kind="ExternalOutput").ap()

    with tile.TileContext(nc) as tc:
        with (
            tc.tile_pool(name="const", bufs=1) as cpool,
            tc.tile_pool(name="gath", bufs=4) as gpool,
            tc.tile_pool(name="work", bufs=3) as wpool,
            tc.tile_pool(name="fin", bufs=2) as fpool,
            tc.tile_pool(name="mpsum", bufs=2, space="PSUM") as mpool,
            tc.tile_pool(name="cpsum", bufs=2, space="PSUM") as clpool,
            tc.tile_pool(name="fpsum", bufs=2, space="PSUM") as fppool,
        ):
            nc.gpsimd.load_library(MLP_LIB)
            # ---- resident SBUF state ----
            zhwt = cpool.tile([D, nwin, W], BF16, tag="zhwt")
            nc.sync.dma_start(out=zhwt[:], in_=zhw[:, :, :])
            cix = cpool.tile([P, cfg.ntok // 16], I16, tag="cix")
            nc.sync.dma_start(out=cix[:], in_=cidx[:, :])
            rlt = cpool.tile([P, cfg.ntok // P], BF16, tag="rlt")
            nc.sync.dma_start(out=rlt[:], in_=rowloc[:, :])
            iot = cpool.tile([P, 1, W], BF16, tag="iot")
            nc.sync.dma_start(out=iot[:, 0, :], in_=iota[:, :])
            wbs = cpool.tile([D + 1, D], F32, tag="wbs")
            nc.sync.dma_start(out=wbs[:], in_=wb[:, :])
            ident = cpool.tile([P, P], F32, tag="ident")
            make_identity(nc, ident[:])
            cb = cpool.tile([P, 1], F32, tag="cb")
            nc.vector.memset(cb[:], -abs(float(alpha)))
            acc = cpool.tile([D, nwin, D + 1], F32, tag="acc")
            nc.vector.memset(acc[:], 0.0)

            # ---- edge phase ----
            for g in range(NGRP):
                tab_g = tabs[g][:, :]
                for st in range(n_stile):
                    s0 = st * STILE                     # slot in group stream
                    tok0 = g * cfg.ntokg + (s0 * P)     # global token base
                    # two 896-token natural gathers -> [128, 14, 128]
                    gt = gpool.tile([P, STILE, DD], BF16, tag="gt")
                    for h in range(2):
                        nc.gpsimd.dma_gather(
                            gt[:, h * (STILE // 2) : (h + 1) * (STILE // 2), :],
                            tab_g,
                            cix[:, (tok0 + h * GB) // 16 : (tok0 + (h + 1) * GB) // 16],
                            GB, GB, DD,
                            queue_num=(st * 2 + h) % 4,
                        )
                    # full on-chip transpose: gtT[elem, s, tok] = gt[tok, s, elem]
                    gtT = wpool.tile([P, STILE, P], BF16, tag="gtT")
                    nc.sync.dma_start_transpose(
                        out=gtT[:], in_=gt[:].rearrange("p s e -> p (s e)"))
                    # aug copy [x | 1]
                    xa = wpool.tile([P, STILE, D + 1], BF16, tag="xa")
                    nc.vector.memset(xa[:, :, D : D + 1], 1.0)
                    nc.scalar.copy(out=xa[:, :, 0:D], in_=gt[:, :, D:DD])
                    # row-eq mask for the super-tile
                    rl = rlt[:, tok0 // P : tok0 // P + STILE]
                    eqm = wpool.tile([P, STILE, W], BF16, tag="eqm")
                    nc.vector.tensor_tensor(
                        out=eqm[:],
                        in0=rl.to_broadcast([P, STILE, W]),
                        in1=iot[:].to_broadcast([P, STILE, W]),
                        op=mybir.AluOpType.is_equal,
                    )
                    # M2 per subchunk; PSUM halves (7 slots = 1792B/bank)
                    ex = wpool.tile([P, STILE, W], BF16, tag="ex")
                    for h in range(2):
                        m2 = mpool.tile([P, STILE // 2, W], F32, tag="m2",
                                        space="PSUM")
                        for si in range(STILE // 2):
                            s = h * (STILE // 2) + si
                            w_id = (s0 + s) // 2
                            nc.tensor.matmul(
                                out=m2[:, si, :],
                                lhsT=gtT[0:D, s, :],
                                rhs=zhwt[:, w_id, :],
                                start=True, stop=True,
                            )
                        nc.scalar.activation(
                            out=ex[:, h * (STILE // 2) : (h + 1) * (STILE // 2), :],
                            in_=m2[:],
                            func=mybir.ActivationFunctionType.Exp,
                            bias=cb[:], scale=float(alpha),
                        )
                    nc.vector.tensor_tensor(
                        out=ex[:], in0=ex[:], in1=eqm[:],
                        op=mybir.AluOpType.mult,
                    )
                    # aggregate: 7 cells x 2 subchunks
                    for c in range(STILE // 2):
                        w_id = (s0 + 2 * c) // 2
                        cell = clpool.tile([W, D + 1], F32, tag="cell",
                                           space="PSUM")
                        for j in range(2):
                            s = 2 * c + j
                            nc.tensor.matmul(
                                out=cell[:],
                                lhsT=ex[:, s, :],
                                rhs=xa[:, s, :],
                                start=(j == 0), stop=(j == 1),
                            )
                        nc.vector.tensor_tensor(
                            out=acc[:, w_id, :], in0=acc[:, w_id, :],
                            in1=cell[:], op=mybir.AluOpType.add,
                        )

            # ---- final phase: out = (msgs/(denom+eps)) @ W^T + b ----
            # repack [64, nwin, 65] -> [128, nwin//2, 65] via SBUF->SBUF DMA
            acc2 = cpool.tile([P, nwin // 2, D + 1], F32, tag="acc2")
            nc.sync.dma_start(out=acc2[0:D, :, :], in_=acc[:, 0::2, :])
            nc.sync.dma_start(out=acc2[D:P, :, :], in_=acc[:, 1::2, :])
            out_v = out.rearrange("(t p) d -> p t d", p=P)
            nt_all = nwin // 2
            for t0 in range(0, nt_all, 4):
                nt = min(4, nt_all - t0)
                a = fpool.tile([P, 4, D + 1], F32, tag="fa")
                dplus = fpool.tile([P, 4], F32, tag="dplus")
                nc.vector.tensor_scalar_add(
                    dplus[:, :nt], acc2[:, t0 : t0 + nt, D : D + 1], EPS_DENOM
                )
                rr = fpool.tile([P, 4], F32, tag="rr")
                nc.vector.reciprocal(out=rr[:, :nt], in_=dplus[:, :nt])
                nc.vector.tensor_tensor(
                    out=a[:, :nt, 0:D], in0=acc2[:, t0 : t0 + nt, 0:D],
                    in1=rr[:, :nt].to_broadcast([P, nt, D]),
                    op=mybir.AluOpType.mult,
                )
                nc.vector.memset(a[:, :nt, D : D + 1], 1.0)
                o = fpool.tile([P, 4, D], F32, tag="fo")
                for i in range(nt):
                    tp = fppool.tile([D + 1, P], F32, tag="tp", space="PSUM")
                    nc.tensor.transpose(out=tp[:], in_=a[:, i, :],
                                        identity=ident[:])
                    lhs = fpool.tile([D + 1, P], F32, tag="lhs")
                    nc.vector.tensor_copy(out=lhs[:], in_=tp[:])
                    y = fppool.tile([P, D], F32, tag="y", space="PSUM")
                    nc.tensor.matmul(out=y[:], lhsT=lhs[:], rhs=wbs[:],
                                     start=True, stop=True)
                    nc.scalar.copy(out=o[:, i, :], in_=y[:])
                nc.sync.dma_start(out=out_v[:, t0 : t0 + nt, :], in_=o[:, :nt, :])

    nc.compile()
    return nc


def _wrap16(a: np.ndarray) -> np.ndarray:
    w = a.reshape(-1, 16).T
    return np.ascontiguousarray(np.tile(w, (8, 1)))


def _pack_core(rows, cols, nwin):
    """Pack one core's edges into the (window, group) cell schedule.

    Returns (win_rows [nwin, W] int32 node-local row ids (-1 junk),
             tok_col int32 [ntok] table-local col (0 for junk),
             tok_rowloc [ntok] float (window-local row slot, 100 junk))
    or None if packing failed.
    """
    grp = cols // GRP
    deg = np.zeros((NPC, NGRP), np.int64)
    np.add.at(deg, (rows, grp), 1)
    order = np.argsort(-deg.max(1), kind="stable")
    loads = np.zeros((nwin, NGRP), np.int64)
    counts = np.zeros(nwin, np.int64)
    assign = np.full(NPC, -1, np.int64)
    slot_of = np.full(NPC, -1, np.int64)
    # greedy best-fit: place heaviest rows first into the most-loaded
    # window that still fits (tightest fit packs best)
    for r in order:
        d = deg[r]
        ok = (counts < W) & np.all(loads + d <= CELL, axis=1)
        if not ok.any():
            return None
        cand = np.where(ok)[0]
        pick = cand[np.argmin((loads[cand] + d).max(1))]
        assign[r] = pick
        slot_of[r] = counts[pick]
        loads[pick] += d
        counts[pick] += 1

    win_rows = np.full((nwin, W), -1, np.int64)
    win_rows[assign, slot_of] = np.arange(NPC)

    ntokg = nwin * 2 * P
    tok_col = np.zeros(nwin * 2 * P * NGRP, np.int64)
    tok_rowloc = np.full(nwin * 2 * P * NGRP, 100.0, np.float32)
    # order edges by (grp, window, arbitrary); place into cell token ranges
    ewin = assign[rows]
    eslot = slot_of[rows]
    # sort by col within each (grp, window) cell: monotone HBM addresses
    o = np.lexsort((cols, ewin, grp))
    # cell base position for each edge: cumulative within (grp, window)
    gs, ws = grp[o], ewin[o]
    new = np.r_[True, (gs[1:] != gs[:-1]) | (ws[1:] != ws[:-1])]
    gid = np.cumsum(new) - 1
    first = np.arange(len(o))[new]
    rank = np.arange(len(o)) - first[gid]
    pos = gs * ntokg + ws * (2 * P) + rank
    tok_col[pos] = cols[o] % GRP
    tok_rowloc[pos] = eslot[o]
    return win_rows, tok_col, tok_rowloc


def shard_inputs(x, z, edge_index, nwin=210):
    z = np.asarray(z, np.float32)
    x = np.asarray(x, np.float32)
    zn = np.maximum(np.sqrt((z * z).sum(1)), 1e-9)
    zh = z / zn[:, None]
    tab = np.ascontiguousarray(
        np.concatenate([zh, x], axis=1).astype(ml_dtypes.bfloat16))

    row = np.asarray(edge_index[0], np.int64)
    col = np.asarray(edge_index[1], np.int64)
    core = row // NPC

    iota = np.tile(np.arange(W, dtype=np.float32), (P, 1)).astype(
        ml_dtypes.bfloat16)

    in_maps = []
    outmaps = []
    for k in range(N_CORES):
        m = core == k
        packed = _pack_core(row[m] % NPC, col[m], nwin)
        if packed is None:
            return None, None, None
        win_rows, tok_col, tok_rowloc = packed
        zhw = np.zeros((D, nwin, W), np.float32)
        valid = win_rows >= 0
        zhw[:, valid] = zh[k * NPC + win_rows[valid]].T
        ntok = tok_col.shape[0]
        in_maps.append({
            **{f"tab{g}": np.ascontiguousarray(tab[g * GRP : (g + 1) * GRP])
               for g in range(NGRP)},
            "zhw": zhw.astype(ml_dtypes.bfloat16),
            "cidx": _wrap16(tok_col.astype(np.int16)),
            "rowloc": np.ascontiguousarray(
                tok_rowloc.reshape(-1, P).T.astype(ml_dtypes.bfloat16)),
            "iota": iota,
        })
        outmaps.append(win_rows)
    return in_maps, outmaps, nwin


def run(x, edge_index, z, Wm, b, alpha, bias_edge, trace=False):
    from concourse.bass_utils import run_bass_kernel_spmd

    nwin = 210
    while True:
        in_maps, outmaps, nwin_used = shard_inputs(x, z, edge_index, nwin)
        if in_maps is not None:
            break
        nwin += 7
    cfg = Cfg(nwin=nwin_used)
    wb = np.ascontiguousarray(
        np.concatenate(
            [np.asarray(Wm, np.float32).T, np.asarray(b, np.float32)[None, :]],
            axis=0,
        )
    )
    for m in in_maps:
        m["wb"] = wb
    nc = build_program(cfg, float(np.asarray(alpha)))
    core_ids = list(range(N_CORES))
    res = run_bass_kernel_spmd(nc, in_maps, core_ids, trace=trace)
    out = assemble_output(res.results, outmaps, cfg)
    return out, res


def assemble_output(results, outmaps, cfg):
    out = np.zeros((N_NODES, D), np.float32)
    for k in range(N_CORES):
        o = np.asarray(results[k]["out"], np.float32)  # [acc_rows, D]
        win_rows = outmaps[k]
        # device row (w, r) interleaved: acc2 put even windows on parts
        # 0:64, odd on 64:128; tile t covers windows (2t, 2t+1);
        # out row index = t*128 + (w%2)*64 + r
        w_idx = np.repeat(np.arange(cfg.nwin), W)
        r_idx = np.tile(np.arange(W), cfg.nwin)
        dev_pos = (w_idx // 2) * P + (w_idx % 2) * W + r_idx
        flat = win_rows.reshape(-1)
        valid = flat >= 0
        out[k * NPC + flat[valid]] = o[dev_pos[valid]]
    return out


def kernel(**inputs) -> np.ndarray:
    out, _ = run(
        inputs["x"], inputs["edge_index"], inputs["z"],
        inputs["W"], inputs["b"], inputs["alpha"], inputs["bias_edge"],
    )
    return out
